# revision 1
# baseline (speedup 1.0000x reference)
"""Trainium2 Bass kernel for nn_ExampleNet (2x NNConv edge-conditioned conv
+ global_add_pool + MLP head), distributed over 8 NeuronCores.

Strategy (edge-parallel, dst-sharded):
  - Host sorts edges by dst node; core k owns dst nodes [6250k, 6250(k+1)).
  - Per core, edges are grouped into 128-node scatter windows; segment-sum is
    done ON-CHIP: per 128-edge tile a one-hot matrix A[e,n]=(dst==n) is built
    with one DVE is_equal op, and a PE matmul with lhsT=A, rhs=tmp accumulates
    the UNREDUCED per-edge products into a PSUM window (node-major), fusing
    the message contraction's i-reduction into a single per-window DVE reduce.
    No indirect scatter, no AllReduce.
  - Per-edge MLP: the big second layer (h @ w2, 97% of FLOPs) runs on PE per
    tile; the tiny first layer h=relu(ea@w1+b1) is host-precomputed edge
    feature prep.  x[src] is host-pre-gathered for conv1.
  - Bias, edge-MLP output bias (b2), and the root term are all folded into
    one per-window root matmul via host-augmented features [x | 1 | deg].
  - h1 node table: AllGather (node-sharded -> replicated); conv2 gathers
    h1[src] rows with per-tile indirect DMAs on the otherwise-idle GPSIMD.
  - global_add_pool: one-hot(batch) matmuls into PSUM; AllReduce of [64,16];
    MLP head replicated on every core.
"""

import sys

sys.path.insert(0, "/opt/trn_rl_repo")

import numpy as np

ml_bf16 = np.float16

import concourse.bass as bass
import concourse.bacc as bacc
import concourse.tile as tile
from concourse import mybir
from concourse import bass_utils
from concourse.bass_interp import get_hw_module

F32 = mybir.dt.float32
I32 = mybir.dt.int32

N_NODES, N_EDGES, N_GRAPHS = 50000, 400000, 64
NF, EF = 16, 8
NCORES = 8
NPC = N_NODES // NCORES          # 6250 nodes owned per core
WIN = 128                        # nodes per scatter window
NWIN = (NPC + WIN - 1) // WIN    # 49 windows per core
NPAD = NWIN * WIN                # 6272 padded nodes per core
P = 128
NT_NODE = NPAD // P              # node-major tiles per core (= NWIN)
PAD_DST = 300.0                  # sentinel: one-hot never matches
CHUNK_T_MAX = 44                 # max tiles per streamed hT chunk
BF16 = mybir.dt.float16  # 16-bit path: fp16 (better mantissa than bf16)

import os
_cache = {}
PROBE_GATHER = os.environ.get("K_PROBE", "0") == "1"
DEBUG_TAPS = False  # add intermediate-dump outputs to the program
SKIP_COLLECTIVES = False  # replace collectives with local DMAs (1-core sim)


# --------------------------------------------------------------------------
# Host-side preparation
# --------------------------------------------------------------------------
def _host_prep(inputs):
    x = np.asarray(inputs["x"], np.float32)
    ei = np.asarray(inputs["edge_index"])
    ea = np.asarray(inputs["edge_attr"], np.float32)
    batch = np.asarray(inputs["batch"]).astype(np.int64)
    src = ei[0].astype(np.int64)
    dst = ei[1].astype(np.int64)

    gw = {k: np.asarray(inputs[k], np.float32) for k in (
        "c1_w1", "c1_b1", "c1_w2", "c1_b2", "c1_root", "c1_bias",
        "c2_w1", "c2_b1", "c2_w2", "c2_b2", "c2_root", "c2_bias",
        "fc1_w", "fc1_b", "out_w", "out_b")}

    # tiny first MLP layers (edge feature prep, host)
    h1e = np.maximum(ea @ gw["c1_w1"] + gw["c1_b1"], 0.0)   # [E, 32]
    h2e = np.maximum(ea @ gw["c2_w1"] + gw["c2_b1"], 0.0)   # [E, 32]
    xs_full = x[src]                                        # [E, 16]
    srcg_full = (src // NPC) * NPAD + (src % NPC)           # gather row ids
    deg = np.bincount(dst, minlength=N_NODES).astype(np.float32)

    # sort edges by dst; contiguous per-core slices
    order = np.argsort(dst, kind="stable")
    dst_s = dst[order]
    core_bounds = np.searchsorted(dst_s, np.arange(NCORES + 1) * NPC)

    # per (core, window) edge counts
    wcnt = np.zeros((NCORES, NWIN), np.int64)
    for k in range(NCORES):
        lo, hi = core_bounds[k], core_bounds[k + 1]
        dl = dst_s[lo:hi] - k * NPC
        wb = np.searchsorted(dl, np.arange(NWIN + 1) * WIN)
        wcnt[k] = np.diff(wb)

    tiles_w = np.maximum(np.ceil(wcnt / P).astype(np.int64).max(axis=0), 0)
    tile_start = np.concatenate([[0], np.cumsum(tiles_w)])
    T = int(tile_start[-1])

    # schedule: chunks of consecutive windows, each <= CHUNK_T_MAX-3
    # tiles; pad each chunk's tile count to a multiple of 4 (for the
    # 4-deep row-tiled We matmul layout)
    chunks = []
    w = 0
    while w < NWIN:
        w2 = w + 1
        while (w2 < NWIN
               and tile_start[w2 + 1] - tile_start[w] <= CHUNK_T_MAX - 3):
            w2 += 1
        pad = (-(tile_start[w2] - tile_start[w])) % 4
        tiles_w[w2 - 1] += pad
        tile_start = np.concatenate([[0], np.cumsum(tiles_w)])
        chunks.append((w, w2))
        w = w2
    T = int(tile_start[-1])
    sched = (T, tuple(int(t) for t in tiles_w), tuple(chunks))

    b2sum1 = gw["c1_b2"].reshape(NF, 32).sum(0)     # [32]
    b2sum2 = gw["c2_b2"].reshape(32, 16).sum(0)     # [16]

    # per-core slot-padded arrays
    per_core = []
    for k in range(NCORES):
        lo, hi = core_bounds[k], core_bounds[k + 1]
        eid = order[lo:hi]
        dl = dst_s[lo:hi] - k * NPC
        wb = np.searchsorted(dl, np.arange(NWIN + 1) * WIN)
        pos = np.arange(hi - lo)
        wof = np.searchsorted(np.arange(1, NWIN + 1) * WIN, dl, side="right")
        slot = (tile_start[wof] * P) + (pos - wb[wof])

        S = T * P
        xs = np.zeros((S, NF), np.float32)
        xs[slot] = xs_full[eid]
        hT1 = np.zeros((32, S), np.float32)
        hT1[:, slot] = h1e[eid].T
        hT2 = np.zeros((32, S), np.float32)
        hT2[:, slot] = h2e[eid].T
        dstrel = np.full(S, PAD_DST, np.float32)
        dstrel[slot] = (dl - wof * WIN).astype(np.float32)
        srcg = np.zeros(S, np.int32)
        srcg[slot] = srcg_full[eid].astype(np.int32)

        # augmented node features for the root matmul: [x | 1 | deg]
        xownT = np.zeros((NF + 2, NPAD), np.float32)
        xownT[:NF, :NPC] = x[k * NPC:(k + 1) * NPC].T
        xownT[NF, :NPC] = 1.0
        xownT[NF + 1, :NPC] = deg[k * NPC:(k + 1) * NPC]
        nconst = np.zeros((2, NPAD), np.float32)     # [1 | deg] rows for h1T
        nconst[0, :NPC] = 1.0
        nconst[1, :NPC] = deg[k * NPC:(k + 1) * NPC]
        blocal = np.full(NPAD, -1.0, np.float32)
        blocal[:NPC] = batch[k * NPC:(k + 1) * NPC].astype(np.float32)

        def stack4(hT):
            # [32, T*128] -> [128, (T//4)*128]: tile t at rows 32*(t%4)
            r = hT.reshape(32, T // 4, 4, P)
            return np.ascontiguousarray(
                r.transpose(2, 0, 1, 3).reshape(P, (T // 4) * P))

        per_core.append(dict(
            xs=np.ascontiguousarray(
                xs.reshape(T, P, NF).transpose(1, 0, 2)
                .reshape(P, T * NF).astype(np.float32)).astype(ml_bf16),
            ht1=stack4(hT1).astype(ml_bf16),
            ht2=stack4(hT2).astype(ml_bf16),
            dstrel=np.ascontiguousarray(dstrel.reshape(T, P).T),
            srcg=np.ascontiguousarray(srcg.reshape(T, P).T),
            xownT=xownT,
            nconst=nconst,
            blocal=np.ascontiguousarray(blocal.reshape(NT_NODE, P).T),
        ))

    # shared weight tensors
    def perm_oi(w2, in_c, out_c):
        # [32, out_c*in_c] in (o,i)-major order
        return np.ascontiguousarray(
            w2.reshape(32, in_c, out_c).transpose(0, 2, 1).reshape(32, -1))

    shared = dict(
        w2a=np.tile(perm_oi(gw["c1_w2"], NF, 32), (4, 1)).astype(ml_bf16),
        w2b=np.tile(perm_oi(gw["c2_w2"], 32, 16), (4, 1)).astype(ml_bf16),
        # root matmul rhs: [root; bias; b2sum]
        root1=np.ascontiguousarray(np.concatenate(
            [gw["c1_root"], gw["c1_bias"][None, :], b2sum1[None, :]], 0)),
        root2=np.ascontiguousarray(np.concatenate(
            [gw["c2_root"], gw["c2_bias"][None, :], b2sum2[None, :]], 0)),
        iota128=np.ascontiguousarray(
            np.broadcast_to(np.arange(WIN, dtype=np.float32),
                            (P, WIN))).astype(ml_bf16),
        iota64=np.ascontiguousarray(
            np.broadcast_to(np.arange(64, dtype=np.float32), (P, 64))),
        ident=np.eye(P, dtype=np.float32),
        fc1w=gw["fc1_w"], fc1b=gw["fc1_b"].reshape(32, 1),
        outw=gw["out_w"], outb=gw["out_b"].reshape(1, 1),
    )
    return sched, per_core, shared


# --------------------------------------------------------------------------
# Device program
# --------------------------------------------------------------------------
def _build_program(sched, reps=1):
    T, tiles_w, chunks = sched
    tile_start = np.concatenate([[0], np.cumsum(tiles_w)]).astype(int)

    nc = bacc.Bacc("TRN2", target_bir_lowering=False, debug=False,
                   enable_asserts=False, num_devices=NCORES,
                   num_swdge_queues=4)

    def din(name, shape, dt=F32):
        return nc.dram_tensor(name, list(shape), dt, kind="ExternalInput").ap()

    xs_d = din("xs", (P, T * NF), BF16)
    ht1_d = din("ht1", (P, (T // 4) * P), BF16)
    ht2_d = din("ht2", (P, (T // 4) * P), BF16)
    dstrel_d = din("dstrel", (P, T))
    srcg_d = din("srcg", (P, T), I32)
    xownT_d = din("xownT", (NF + 2, NPAD))
    nconst_d = din("nconst", (2, NPAD))
    blocal_d = din("blocal", (P, NT_NODE))
    w2a_d = din("w2a", (P, 512), BF16)
    w2b_d = din("w2b", (P, 512), BF16)
    root1_d = din("root1", (NF + 2, 32))
    root2_d = din("root2", (34, 16))
    iota128_d = din("iota128", (P, WIN), BF16)
    iota64_d = din("iota64", (P, 64))
    ident_d = din("ident", (P, P))
    fc1w_d = din("fc1w", (NF, 32))
    fc1b_d = din("fc1b", (32, 1))
    outw_d = din("outw", (32, 1))
    outb_d = din("outb", (1, 1))
    y_d = nc.dram_tensor("y", [1, 64], F32, kind="ExternalOutput").ap()
    probe_d = (nc.dram_tensor("t_probe", [P, 4 * 32], F32,
                              kind="ExternalOutput").ap()
               if PROBE_GATHER else None)
    taps = {}
    if DEBUG_TAPS:
        for nm, shape in [("t_h1nm", (P, NT_NODE * 32)),
                          ("t_h1s0", (P, 32)), ("t_g", (64, 16))]:
            taps[nm] = nc.dram_tensor(nm, list(shape), F32,
                                      kind="ExternalOutput").ap()

    with tile.TileContext(nc) as tc:
        with (
            tc.tile_pool(name="const", bufs=1) as cp,
            tc.tile_pool(name="stream", bufs=2) as sp,
            tc.tile_pool(name="work", bufs=4) as wp,
            tc.tile_pool(name="psum", bufs=2, space="PSUM") as pp,
            tc.tile_pool(name="psum_agg", bufs=2, space="PSUM") as pagg,
            tc.tile_pool(name="dram", bufs=1, space="DRAM") as dp,
        ):
            # ---- persistent SBUF loads
            def load(dram_ap, shape, dt=F32, tag=None):
                t = cp.tile(list(shape), dt, tag=tag)
                nc.sync.dma_start(t[:], dram_ap)
                return t

            xs_s = load(xs_d, (P, T * NF), BF16, tag="xs_s")
            dstrel_s = load(dstrel_d, (P, T), tag="dstrel_s")
            srcg_s = load(srcg_d, (P, T), I32, tag="srcg_s")
            xownT_s = load(xownT_d, (NF + 2, NPAD), tag="xownT_s")
            blocal_s = load(blocal_d, (P, NT_NODE), tag="blocal_s")
            w2a_s = load(w2a_d, (P, 512), BF16, tag="w2a_s")
            w2b_s = load(w2b_d, (P, 512), BF16, tag="w2b_s")
            root1_s = load(root1_d, (NF + 2, 32), tag="root1_s")
            root2_s = load(root2_d, (34, 16), tag="root2_s")
            iota128_s = load(iota128_d, (P, WIN), BF16, tag="iota128_s")
            iota64_s = load(iota64_d, (P, 64), tag="iota64_s")
            ident_s = load(ident_d, (P, P), tag="ident_s")
            fc1w_s = load(fc1w_d, (NF, 32), tag="fc1w_s")
            fc1b_s = load(fc1b_d, (32, 1), tag="fc1b_s")
            outw_s = load(outw_d, (32, 1), tag="outw_s")
            outb_s = load(outb_d, (1, 1), tag="outb_s")

            # node tables (node-major) + feature-major h1 for conv2 root
            h1nm = cp.tile([P, NT_NODE * 32], F32, tag="h1nm")
            h2nm = cp.tile([P, NT_NODE * 16], F32, tag="h2nm")
            h1T = cp.tile([34, NPAD], F32, tag="h1T")
            nc.sync.dma_start(h1T[32:34, :], nconst_d)
            # one-hot A tiles are identical across the two conv layers
            # (same dstrel): conv1 builds them once into a persistent
            # cache; conv2 reuses.
            Acache = cp.tile([P, T * WIN], BF16, tag="Acache")
            zeros32 = cp.tile([P, 32], F32, tag="zeros32")
            nc.vector.memset(zeros32[:], 0.0)


            # ------------------------------------------------------------
            def conv_layer(ht_d, w2_s, in_c, out_c, root_lhsT, root_rhs,
                           hout_nm, src_view, build_A):
                """One NNConv layer; writes node-major relu output into
                hout_nm ([P, NT_NODE*out_c], window w at cols [w*out_c:])."""
                for (wlo, whi) in chunks:
                    clo, chi = tile_start[wlo], tile_start[whi]
                    ct = chi - clo
                    if ct > 0:
                        ht_c = sp.tile([P, (CHUNK_T_MAX // 4) * P], BF16,
                                       tag="ht_c")
                        nc.sync.dma_start(
                            ht_c[:, :(ct // 4) * P],
                            ht_d[:, (clo // 4) * P:(chi // 4) * P])
                    for w in range(wlo, whi):
                        nw = int(tiles_w[w])
                        root_ps = pp.tile([P, out_c], F32, tag="aux")
                        nc.tensor.matmul(
                            root_ps[:], lhsT=root_lhsT[:, w * WIN:(w + 1) * WIN],
                            rhs=root_rhs, start=True, stop=True)
                        if nw > 0:
                            unred = pagg.tile([P, 512], F32, tag="unred")
                        for ti in range(nw):
                            t = int(tile_start[w]) + ti
                            tl = t - clo
                            # We = hT.T @ w2 -> [128e, 512] PSUM
                            # (4x row-tiled: tile t uses PE rows 32*(t%4))
                            g4 = t % 4
                            we = pp.tile([P, 512], F32, tag="we", bufs=4)
                            nc.tensor.matmul(
                                we[:],
                                lhsT=ht_c[32 * g4:32 * (g4 + 1),
                                          (tl // 4) * P:(tl // 4 + 1) * P],
                                rhs=w2_s[32 * g4:32 * (g4 + 1), :],
                                start=True, stop=True,
                                tile_position=(32 * g4, 0))
                            # evacuate+cast We -> SBUF bf16 (on ACT)
                            we_sb = wp.tile([P, 512], BF16, tag="we_sb",
                                            bufs=4)
                            nc.scalar.activation(
                                out=we_sb[:], in_=we[:],
                                func=mybir.ActivationFunctionType.Copy)
                            # one-hot A [128, WIN] bf16 (cached
                            # across layers)
                            A = Acache[:, t * WIN:(t + 1) * WIN]
                            if build_A:
                                nc.vector.tensor_scalar(
                                    out=A, in0=iota128_s[:],
                                    scalar1=dstrel_s[:, t:t + 1],
                                    scalar2=None,
                                    op0=mybir.AluOpType.is_equal)
                            # tmp = xs_bcast * We   [128, out_c, in_c] bf16
                            tmp = wp.tile([P, out_c, in_c], BF16,
                                          tag="tmp", bufs=3)
                            we3 = we_sb[:].rearrange(
                                "p (o i) -> p o i", o=out_c, i=in_c)
                            src_b = src_view(t, tl)[:, None, :] \
                                .broadcast_to([P, out_c, in_c])
                            nc.vector.tensor_tensor(
                                out=tmp[:], in0=we3, in1=src_b,
                                op=mybir.AluOpType.mult)
                            # scatter UNREDUCED: unred += A.T @ tmp
                            nc.tensor.matmul(
                                unred[:], lhsT=A,
                                rhs=tmp[:].rearrange("p o i -> p (o i)"),
                                start=(ti == 0), stop=(ti == nw - 1))
                        # combine: h = relu(reduce_i(unred) + root)
                        ocol = slice(w * out_c, (w + 1) * out_c)
                        if nw > 0:
                            r = wp.tile([P, out_c], F32, tag="r")
                            nc.vector.tensor_reduce(
                                out=r[:],
                                in_=unred[:].rearrange(
                                    "p (o i) -> p o i", o=out_c, i=in_c),
                                axis=mybir.AxisListType.X,
                                op=mybir.AluOpType.add)
                            s = wp.tile([P, out_c], F32, tag="s")
                            nc.vector.tensor_tensor(
                                out=s[:], in0=r[:], in1=root_ps[:],
                                op=mybir.AluOpType.add)
                        else:
                            s = wp.tile([P, out_c], F32, tag="s")
                            nc.vector.tensor_copy(s[:], root_ps[:])
                        nc.vector.tensor_tensor(
                            out=hout_nm[:, ocol], in0=s[:],
                            in1=zeros32[:, :out_c],
                            op=mybir.AluOpType.max)

            def run_once(rep):
                # DRAM internals for collectives (fresh per rep: Shared
                # DRAM may only be written by a single instruction)
                ag_in = dp.tile([NPAD, 32], BF16, tag=f"ag_in{rep}")
                ag_out = dp.tile([NCORES * NPAD, 32], BF16,
                                 tag=f"ag_out{rep}", addr_space="Shared")
                ar_in = dp.tile([64, 16], F32, tag=f"ar_in{rep}")
                ar_out = dp.tile([64, 16], F32, tag=f"ar_out{rep}",
                                 addr_space="Shared")
                # ---- conv1
                conv_layer(ht1_d, w2a_s, NF, 32, xownT_s[:], root1_s[:], h1nm,
                           lambda t, tl: xs_s[:, t * NF:(t + 1) * NF],
                           build_A=(rep == 0))

                # ---- ship h1 (node-major) to AllGather; build feature-major h1T
                nc.gpsimd.dma_start(
                    ag_in[:].rearrange("(t p) f -> p t f", p=P),
                    h1nm[:].rearrange("p (t f) -> p t f", f=32))
                if SKIP_COLLECTIVES:
                    nc.sync.dma_start(ag_out[:NPAD, :], ag_in[:])
                else:
                    nc.gpsimd.collective_compute(
                        "AllGather", mybir.AluOpType.bypass,
                        replica_groups=[list(range(NCORES))],
                        ins=[ag_in[:].opt()], outs=[ag_out[:].opt()])
                for nt in range(NT_NODE):
                    tp = pp.tile([32, P], F32, tag="aux")
                    nc.tensor.transpose(
                        tp[:], in_=h1nm[:, nt * 32:(nt + 1) * 32],
                        identity=ident_s[:, :])
                    nc.vector.tensor_copy(
                        h1T[:32, nt * P:(nt + 1) * P], tp[:])
                if DEBUG_TAPS:
                    nc.sync.dma_start(taps["t_h1nm"], h1nm[:])

                # ---- conv2: gather h1[src] per tile (canonical [128,1]-offset
                # indirect DMA on GPSIMD; overlaps PE/DVE compute)
                if PROBE_GATHER and rep == 0:
                    pg = wp.tile([P, 4, 32], BF16, tag="probe_g", bufs=1)
                    nc.gpsimd.indirect_dma_start(
                        out=pg[:], out_offset=None, in_=ag_out[:],
                        in_offset=bass.IndirectOffsetOnAxis(
                            ap=srcg_s[:, 0:4], axis=0),
                    )
                    pgf = wp.tile([P, 4 * 32], F32, tag="probe_f", bufs=1)
                    nc.vector.tensor_copy(
                        pgf[:], pg[:].rearrange("p a b -> p (a b)"))
                    nc.sync.dma_start(probe_d, pgf[:])

                def h1s_tile(t, tl):
                    g = wp.tile([P, 32], BF16, tag="h1s_t", bufs=12)
                    inst = nc.gpsimd.indirect_dma_start(
                        out=g[:], out_offset=None, in_=ag_out[:],
                        in_offset=bass.IndirectOffsetOnAxis(
                            ap=srcg_s[:, t:t + 1], axis=0),
                    )
                    if t % 4:
                        inst.queue = "qPoolDynamic%d" % (t % 4)
                    return g[:]

                conv_layer(ht2_d, w2b_s, 32, 16, h1T[:], root2_s[:], h2nm,
                           h1s_tile, build_A=False)

                # ---- global_add_pool: one-hot(batch) matmuls, node-major h2
                g_ps = pp.tile([64, 16], F32, tag="aux")
                for nt in range(NT_NODE):
                    B = wp.tile([P, 64], F32, tag="B", bufs=2)
                    nc.vector.tensor_scalar(
                        out=B[:], in0=iota64_s[:],
                        scalar1=blocal_s[:, nt:nt + 1], scalar2=None,
                        op0=mybir.AluOpType.is_equal)
                    nc.tensor.matmul(
                        g_ps[:], lhsT=B[:], rhs=h2nm[:, nt * 16:(nt + 1) * 16],
                        start=(nt == 0), stop=(nt == NT_NODE - 1))
                g_s = wp.tile([64, 16], F32, tag="g_s")
                nc.vector.tensor_copy(g_s[:], g_ps[:])
                if DEBUG_TAPS:
                    nc.sync.dma_start(taps["t_g"], g_s[:])
                nc.sync.dma_start(ar_in[:], g_s[:])
                if SKIP_COLLECTIVES:
                    nc.sync.dma_start(ar_out[:], ar_in[:])
                else:
                    nc.gpsimd.collective_compute(
                        "AllReduce", mybir.AluOpType.add,
                        replica_groups=[list(range(NCORES))],
                        ins=[ar_in[:].opt()], outs=[ar_out[:].opt()])
                g_r = wp.tile([64, 16], F32, tag="g_r")
                nc.sync.dma_start(g_r[:], ar_out[:])

                # ---- head: y = relu(g@fc1+b)@out_w + out_b
                gT_ps = pp.tile([16, 64], F32, tag="aux")
                nc.tensor.transpose(gT_ps[:], in_=g_r[:], identity=ident_s[:64, :64])
                gT_s = wp.tile([16, 64], F32, tag="gT_s")
                nc.vector.tensor_copy(gT_s[:], gT_ps[:])
                o1 = pp.tile([32, 64], F32, tag="aux")
                nc.tensor.matmul(o1[:], lhsT=fc1w_s[:], rhs=gT_s[:],
                                 start=True, stop=True)
                r1 = wp.tile([32, 64], F32, tag="r1")
                nc.scalar.activation(out=r1[:], in_=o1[:],
                                     func=mybir.ActivationFunctionType.Relu,
                                     bias=fc1b_s[:])
                o2 = pp.tile([1, 64], F32, tag="aux")
                nc.tensor.matmul(o2[:], lhsT=outw_s[:], rhs=r1[:],
                                 start=True, stop=True)
                ys = wp.tile([1, 64], F32, tag="ys")
                nc.vector.tensor_scalar(out=ys[:], in0=o2[:],
                                        scalar1=outb_s[:], scalar2=None,
                                        op0=mybir.AluOpType.add)
                nc.sync.dma_start(y_d, ys[:])

            for _rep in range(reps):
                run_once(_rep)

    nc.compile()
    nc.m = get_hw_module(nc.m)
    return nc


# --------------------------------------------------------------------------
def kernel(**inputs):
    sched, per_core, shared = _host_prep(inputs)
    key = sched
    if key not in _cache:
        _cache[key] = _build_program(sched)
    nc = _cache[key]

    in_maps = []
    for k in range(NCORES):
        m = dict(shared)
        m.update(per_core[k])
        m = {n: np.ascontiguousarray(v) for n, v in m.items()}
        in_maps.append(m)

    res = bass_utils.run_bass_kernel_spmd(nc, in_maps,
                                          core_ids=list(range(NCORES)))
    y = np.asarray(res.results[0]["y"], np.float32).reshape(64, 1)
    return y




# revision 68
# speedup vs baseline: 11.3173x; 11.3173x over previous
"""Trainium2 Bass kernel for nn_ExampleNet (2x NNConv edge-conditioned conv
+ global_add_pool + MLP head), distributed over 8 NeuronCores.

Strategy (edge-parallel, dst-sharded):
  - Host sorts edges by dst node; core k owns dst nodes [6250k, 6250(k+1)).
  - Per core, edges are grouped into 128-node scatter windows; segment-sum is
    done ON-CHIP: per 128-edge tile a one-hot matrix A[e,n]=(dst==n) is built
    with one DVE is_equal op (cached in SBUF across layers/reps), and a PE
    matmul with lhsT=A, rhs=tmp accumulates the UNREDUCED per-edge products
    into a PSUM window (node-major), fusing the message contraction's
    i-reduction into a single per-window DVE reduce.
  - Per-edge MLP: the big second layer (h @ w2, 97% of FLOPs) runs on PE per
    tile; the tiny first layer h=relu(ea@w1+b1) is host-precomputed edge
    feature prep.  x[src] is host-pre-gathered and streamed per chunk.
  - We evacuation is load-balanced: most tiles evac PSUM->SBUF-bf16 on ACT
    then multiply on DVE; every DVE_DIRECT_MOD-th tile instead has DVE read
    We straight from PSUM (fused evac+mult), keeping both engines busy.
  - Root terms (+biases, via host-augmented features [x | 1 | deg]) are
    batch-precomputed into node-major SBUF tables: conv1's once (static
    across reps), conv2's per rep during the AllGather dead time.
  - h1 node table: CHUNKED AllGather fired as conv1 windows complete
    (overlaps the collective); chunks land in a 256B-padded DRAM table.
    conv2 fetches h1[src] with one dma_gather per 7-tile group: int16
    indices rebased to the table midpoint (signed offsets cover all 50176
    rows), <=896 descriptors per op (SWDGE ring limit ~1024), group tails
    kept non-negative by host-side slot swaps (trailing negatives drop).
  - global_add_pool is fused into conv2's window loop (one-hot(batch)
    matmul + SBUF accumulate per window); the cross-core reduction is an
    AllGather of [64,16] + on-chip sum (cheaper than AllReduce); MLP head
    replicated on every core.
"""

import sys

sys.path.insert(0, "/opt/trn_rl_repo")

import numpy as np

ml_bf16 = np.float16

import concourse.bass as bass
import concourse.bacc as bacc
import concourse.tile as tile
from concourse import mybir
from concourse import bass_utils
from concourse.bass_interp import get_hw_module

F32 = mybir.dt.float32
I32 = mybir.dt.int32

N_NODES, N_EDGES, N_GRAPHS = 50000, 400000, 64
NF, EF = 16, 8
NCORES = 8
NPC = N_NODES // NCORES          # 6250 nodes owned per core
WIN = 128                        # nodes per scatter window
NWIN = (NPC + WIN - 1) // WIN    # 49 windows per core
NPAD = NWIN * WIN                # 6272 padded nodes per core
P = 128
NT_NODE = NPAD // P              # node-major tiles per core (= NWIN)
PAD_DST = 300.0                  # sentinel: one-hot never matches
CHUNK_T_MAX = 44                 # max tiles per streamed hT chunk
BF16 = mybir.dt.float16  # 16-bit path: fp16 (better mantissa than bf16)
# AllGather chunk boundaries (window indices): AG of h1 for windows
# [AG_SPLITS[c], AG_SPLITS[c+1]) fires as soon as those windows complete,
# overlapping the collective with conv1 compute.
AG_SPLITS = (0, 16, 30, 42, NWIN)
DVE_DIRECT_MOD = 5               # every k-th tile: DVE reads We from PSUM
                                 # directly (fused evac+mult), offloading ACT
USE_NEW_TAIL = True
GATHER_G = 7                     # tiles per dma_gather group: 896 descs,
                                 # under the ~1024-desc SWDGE ring capacity
TBL_BASE = (NCORES * NPAD) // 2  # signed-int16 rebase point for gather idxs
TBL_ROW = 128                    # padded table row: 128 bf16 = 256B


def _gather_groups(sched):
    """Deterministic (host+device) split of each chunk's tiles into
    dma_gather groups of <= GATHER_G tiles. Returns list of
    (tile_lo, tile_hi, col_lo) where col_lo indexes srcg16 columns."""
    T, tiles_w, chunks = sched
    tile_start = np.concatenate([[0], np.cumsum(tiles_w)]).astype(int)
    groups = []
    col = 0
    for (wlo, whi) in chunks:
        clo, chi = int(tile_start[wlo]), int(tile_start[whi])
        for g0 in range(clo, chi, GATHER_G):
            g1 = min(g0 + GATHER_G, chi)
            groups.append((g0, g1, col))
            col += (g1 - g0) * 8   # 128 idx/tile wrapped into 16 rows
    return groups, col

import os
_cache = {}
PROBE_GATHER = os.environ.get("K_PROBE", "0") == "1"
DEBUG_TAPS = False  # add intermediate-dump outputs to the program
SKIP_COLLECTIVES = False  # replace collectives with local DMAs (1-core sim)


# --------------------------------------------------------------------------
# Host-side preparation
# --------------------------------------------------------------------------
def _host_prep(inputs):
    x = np.asarray(inputs["x"], np.float32)
    ei = np.asarray(inputs["edge_index"])
    ea = np.asarray(inputs["edge_attr"], np.float32)
    batch = np.asarray(inputs["batch"]).astype(np.int64)
    src = ei[0].astype(np.int64)
    dst = ei[1].astype(np.int64)

    gw = {k: np.asarray(inputs[k], np.float32) for k in (
        "c1_w1", "c1_b1", "c1_w2", "c1_b2", "c1_root", "c1_bias",
        "c2_w1", "c2_b1", "c2_w2", "c2_b2", "c2_root", "c2_bias",
        "fc1_w", "fc1_b", "out_w", "out_b")}

    # tiny first MLP layers (edge feature prep, host)
    h1e = np.maximum(ea @ gw["c1_w1"] + gw["c1_b1"], 0.0)   # [E, 32]
    h2e = np.maximum(ea @ gw["c2_w1"] + gw["c2_b1"], 0.0)   # [E, 32]
    xs_full = x[src]                                        # [E, 16]
    # gather row ids into the chunk-major AllGather output layout:
    # ag_out rows = [chunk c][core k][local row l - lo_c]
    ag_lo = np.asarray(AG_SPLITS[:-1]) * WIN
    ag_hi = np.asarray(AG_SPLITS[1:]) * WIN
    ag_rows = ag_hi - ag_lo
    ag_base = np.concatenate([[0], np.cumsum(NCORES * ag_rows)])
    src_k = src // NPC
    src_l = src % NPC
    src_c = np.searchsorted(ag_hi, src_l, side="right")
    srcg_full = (ag_base[src_c] + src_k * ag_rows[src_c]
                 + (src_l - ag_lo[src_c]))
    deg = np.bincount(dst, minlength=N_NODES).astype(np.float32)

    # sort edges by dst; contiguous per-core slices
    order = np.argsort(dst, kind="stable")
    dst_s = dst[order]
    core_bounds = np.searchsorted(dst_s, np.arange(NCORES + 1) * NPC)

    # per (core, window) edge counts
    wcnt = np.zeros((NCORES, NWIN), np.int64)
    for k in range(NCORES):
        lo, hi = core_bounds[k], core_bounds[k + 1]
        dl = dst_s[lo:hi] - k * NPC
        wb = np.searchsorted(dl, np.arange(NWIN + 1) * WIN)
        wcnt[k] = np.diff(wb)

    tiles_w = np.maximum(np.ceil(wcnt / P).astype(np.int64).max(axis=0), 0)
    tile_start = np.concatenate([[0], np.cumsum(tiles_w)])
    T = int(tile_start[-1])

    # schedule: chunks of consecutive windows, each <= CHUNK_T_MAX-3
    # tiles; pad each chunk's tile count to a multiple of 4 (for the
    # 4-deep row-tiled We matmul layout)
    chunks = []
    w = 0
    while w < NWIN:
        w2 = w + 1
        while (w2 < NWIN
               and tile_start[w2 + 1] - tile_start[w] <= CHUNK_T_MAX - 3):
            w2 += 1
        pad = (-(tile_start[w2] - tile_start[w])) % 4
        tiles_w[w2 - 1] += pad
        tile_start = np.concatenate([[0], np.cumsum(tiles_w)])
        chunks.append((w, w2))
        w = w2
    T = int(tile_start[-1])
    sched = (T, tuple(int(t) for t in tiles_w), tuple(chunks))

    b2sum1 = gw["c1_b2"].reshape(NF, 32).sum(0)     # [32]
    b2sum2 = gw["c2_b2"].reshape(32, 16).sum(0)     # [16]

    # per-core slot-padded arrays
    per_core = []
    for k in range(NCORES):
        lo, hi = core_bounds[k], core_bounds[k + 1]
        eid = order[lo:hi]
        dl = dst_s[lo:hi] - k * NPC
        wb = np.searchsorted(dl, np.arange(NWIN + 1) * WIN)
        pos = np.arange(hi - lo)
        wof = np.searchsorted(np.arange(1, NWIN + 1) * WIN, dl, side="right")
        slot = (tile_start[wof] * P) + (pos - wb[wof])

        S = T * P
        xs = np.zeros((S, NF), np.float32)
        xs[slot] = xs_full[eid]
        hT1 = np.zeros((32, S), np.float32)
        hT1[:, slot] = h1e[eid].T
        hT2 = np.zeros((32, S), np.float32)
        hT2[:, slot] = h2e[eid].T
        dstrel = np.full(S, PAD_DST, np.float32)
        dstrel[slot] = (dl - wof * WIN).astype(np.float32)
        # padded slots default to TBL_BASE (rebased idx 0: harmless row,
        # and keeps group tails non-negative -- trailing negatives DROP)
        srcg = np.full(S, TBL_BASE, np.int64)
        srcg[slot] = srcg_full[eid]

        # tile t belongs to window w iff tile_start[w] <= t < tile_start[w+1]
        tile_win = np.searchsorted(tile_start[1:], np.arange(T),
                                   side="right")
        slot_win = tile_win[np.arange(S) // P]

        groups, ncols = _gather_groups(sched)

        # ensure no gather group ENDS on a negative rebased idx: swap the
        # offending last slot with an idx>=TBL_BASE slot in the same window
        def swap_slots(a, b):
            xs[[a, b]] = xs[[b, a]]
            hT1[:, [a, b]] = hT1[:, [b, a]]
            hT2[:, [a, b]] = hT2[:, [b, a]]
            dstrel[[a, b]] = dstrel[[b, a]]
            srcg[[a, b]] = srcg[[b, a]]

        last_slots = {g1 * P - 1 for (_g0, g1, _c) in groups}
        for (g0, g1, col) in groups:
            last = g1 * P - 1
            if srcg[last] >= TBL_BASE:
                continue
            w = slot_win[last]
            wlo_s = int(tile_start[w]) * P
            whi_s = int(tile_start[w + 1]) * P
            cand = np.nonzero(srcg[wlo_s:whi_s] >= TBL_BASE)[0]
            cand = [wlo_s + c for c in cand
                    if (wlo_s + c) not in last_slots]
            assert cand, "no non-negative idx slot to swap into group tail"
            swap_slots(last, cand[0])

        srcg16 = np.zeros((16, ncols), np.int16)
        for (g0, g1, col) in groups:
            n = (g1 - g0) * P
            seg = (srcg[g0 * P:g1 * P] - TBL_BASE).astype(np.int16)
            i = np.arange(n)
            srcg16[i % 16, col + i // 16] = seg
        srcg16 = np.tile(srcg16, (8, 1))

        # augmented node features for the root matmul: [x | 1 | deg]
        xownT = np.zeros((NF + 2, NPAD), np.float32)
        xownT[:NF, :NPC] = x[k * NPC:(k + 1) * NPC].T
        xownT[NF, :NPC] = 1.0
        xownT[NF + 1, :NPC] = deg[k * NPC:(k + 1) * NPC]
        xownT = xownT.astype(ml_bf16)
        nconst = np.zeros((2, NPAD), np.float32)     # [1 | deg] rows for h1T
        nconst[0, :NPC] = 1.0
        nconst[1, :NPC] = deg[k * NPC:(k + 1) * NPC]
        nconst = nconst.astype(ml_bf16)
        blocal = np.full(NPAD, -1.0, np.float32)
        blocal[:NPC] = batch[k * NPC:(k + 1) * NPC].astype(np.float32)

        def stack4(hT):
            # [32, T*128] -> [128, (T//4)*128]: tile t at rows 32*(t%4)
            r = hT.reshape(32, T // 4, 4, P)
            return np.ascontiguousarray(
                r.transpose(2, 0, 1, 3).reshape(P, (T // 4) * P))

        per_core.append(dict(
            xs=np.ascontiguousarray(
                xs.reshape(T, P, NF).transpose(1, 0, 2)
                .reshape(P, T * NF).astype(np.float32)).astype(ml_bf16),
            ht1=stack4(hT1).astype(ml_bf16),
            ht2=stack4(hT2).astype(ml_bf16),
            dstrel=np.ascontiguousarray(dstrel.reshape(T, P).T),
            srcg16=np.ascontiguousarray(srcg16),
            xownT=xownT,
            nconst=nconst,
            blocal=np.ascontiguousarray(blocal.reshape(NT_NODE, P).T),
        ))

    # shared weight tensors
    def perm_oi(w2, in_c, out_c):
        # [32, out_c*in_c] in (o,i)-major order
        return np.ascontiguousarray(
            w2.reshape(32, in_c, out_c).transpose(0, 2, 1).reshape(32, -1))

    shared = dict(
        w2a=np.tile(perm_oi(gw["c1_w2"], NF, 32), (4, 1)).astype(ml_bf16),
        w2b=np.tile(perm_oi(gw["c2_w2"], 32, 16), (4, 1)).astype(ml_bf16),
        # root matmul rhs: [root; bias; b2sum]
        root1=np.ascontiguousarray(np.concatenate(
            [gw["c1_root"], gw["c1_bias"][None, :], b2sum1[None, :]],
            0)).astype(ml_bf16),
        root2=np.ascontiguousarray(np.concatenate(
            [gw["c2_root"], gw["c2_bias"][None, :], b2sum2[None, :]],
            0)).astype(ml_bf16),
        iota128=np.ascontiguousarray(
            np.broadcast_to(np.arange(WIN, dtype=np.float32),
                            (P, WIN))).astype(ml_bf16),
        iota64=np.ascontiguousarray(
            np.broadcast_to(np.arange(64, dtype=np.float32), (P, 64))),
        ident=np.eye(P, dtype=np.float32),
        fc1w=gw["fc1_w"], fc1b=gw["fc1_b"].reshape(32, 1),
        outw=gw["out_w"], outb=gw["out_b"].reshape(1, 1),
    )
    return sched, per_core, shared


# --------------------------------------------------------------------------
# Device program
# --------------------------------------------------------------------------
def _build_program(sched, reps=1):
    T, tiles_w, chunks = sched
    tile_start = np.concatenate([[0], np.cumsum(tiles_w)]).astype(int)

    nc = bacc.Bacc("TRN2", target_bir_lowering=False, debug=False,
                   enable_asserts=False, num_devices=NCORES,
                   num_swdge_queues=4)

    def din(name, shape, dt=F32):
        return nc.dram_tensor(name, list(shape), dt, kind="ExternalInput").ap()

    groups, g_ncols = _gather_groups(sched)
    xs_d = din("xs", (P, T * NF), BF16)
    ht1_d = din("ht1", (P, (T // 4) * P), BF16)
    ht2_d = din("ht2", (P, (T // 4) * P), BF16)
    dstrel_d = din("dstrel", (P, T))
    srcg16_d = din("srcg16", (P, g_ncols), mybir.dt.int16)
    xownT_d = din("xownT", (NF + 2, NPAD), BF16)
    nconst_d = din("nconst", (2, NPAD), BF16)
    blocal_d = din("blocal", (P, NT_NODE))
    w2a_d = din("w2a", (P, 512), BF16)
    w2b_d = din("w2b", (P, 512), BF16)
    root1_d = din("root1", (NF + 2, 32), BF16)
    root2_d = din("root2", (34, 16), BF16)
    iota128_d = din("iota128", (P, WIN), BF16)
    iota64_d = din("iota64", (P, 64))
    ident_d = din("ident", (P, P))
    fc1w_d = din("fc1w", (NF, 32))
    fc1b_d = din("fc1b", (32, 1))
    outw_d = din("outw", (32, 1))
    outb_d = din("outb", (1, 1))
    y_d = nc.dram_tensor("y", [1, 64], F32, kind="ExternalOutput").ap()
    taps = {}
    if DEBUG_TAPS:
        for nm, shape in [("t_h1nm", (P, NT_NODE * 32)),
                          ("t_h1s0", (P, 32)), ("t_g", (64, 16))]:
            taps[nm] = nc.dram_tensor(nm, list(shape), F32,
                                      kind="ExternalOutput").ap()

    with tile.TileContext(nc) as tc:
        with (
            tc.tile_pool(name="const", bufs=1) as cp,
            tc.tile_pool(name="stream", bufs=2) as sp,
            tc.tile_pool(name="work", bufs=4) as wp,
            tc.tile_pool(name="psum", bufs=2, space="PSUM") as pp,
            tc.tile_pool(name="psum_we", bufs=2, space="PSUM") as ppw,
            tc.tile_pool(name="psum_agg", bufs=2, space="PSUM") as pagg,
            tc.tile_pool(name="dram", bufs=1, space="DRAM") as dp,
        ):
            # ---- persistent SBUF loads
            def load(dram_ap, shape, dt=F32, tag=None):
                t = cp.tile(list(shape), dt, tag=tag)
                nc.sync.dma_start(t[:], dram_ap)
                return t

            dstrel_s = load(dstrel_d, (P, T), tag="dstrel_s")
            srcg16_s = load(srcg16_d, (P, g_ncols), mybir.dt.int16,
                            tag="srcg16_s")
            xownT_s = load(xownT_d, (NF + 2, NPAD), BF16, tag="xownT_s")
            blocal_s = load(blocal_d, (P, NT_NODE), tag="blocal_s")
            w2a_s = load(w2a_d, (P, 512), BF16, tag="w2a_s")
            w2b_s = load(w2b_d, (P, 512), BF16, tag="w2b_s")
            root1_s = load(root1_d, (NF + 2, 32), BF16, tag="root1_s")
            root2_s = load(root2_d, (34, 16), BF16, tag="root2_s")
            iota128_s = load(iota128_d, (P, WIN), BF16, tag="iota128_s")
            iota64_s = load(iota64_d, (P, 64), tag="iota64_s")
            ident_s = load(ident_d, (P, P), tag="ident_s")
            fc1w_s = load(fc1w_d, (NF, 32), tag="fc1w_s")
            fc1b_s = load(fc1b_d, (32, 1), tag="fc1b_s")
            outw_s = load(outw_d, (32, 1), tag="outw_s")
            outb_s = load(outb_d, (1, 1), tag="outb_s")

            # node tables (node-major) + feature-major h1 for conv2 root
            h1nm = cp.tile([P, NT_NODE * 32], F32, tag="h1nm")
            h2nm = cp.tile([P, NT_NODE * 16], F32, tag="h2nm")
            h1T = cp.tile([34, NPAD], BF16, tag="h1T")
            nc.sync.dma_start(h1T[32:34, :], nconst_d)
            # one-hot A tiles are identical across the two conv layers
            # (same dstrel): conv1 builds them once into a persistent
            # cache; conv2 reuses.
            Acache = cp.tile([P, T * WIN], BF16, tag="Acache")
            # node-major root tables: root1nm is static across reps (f(x));
            # root2nm recomputed per rep from h1T (overlaps the AllGather)
            root1nm = cp.tile([P, NT_NODE * 32], BF16, tag="root1nm")
            root2nm = cp.tile([P, NT_NODE * 16], BF16, tag="root2nm")
            g_acc = cp.tile([64, 16], F32, tag="g_acc")
            for nt in range(NT_NODE):
                rps = pp.tile([P, 32], F32, tag="aux")
                nc.tensor.matmul(
                    rps[:], lhsT=xownT_s[:, nt * WIN:(nt + 1) * WIN],
                    rhs=root1_s[:], start=True, stop=True)
                nc.scalar.activation(
                    out=root1nm[:, nt * 32:(nt + 1) * 32], in_=rps[:],
                    func=mybir.ActivationFunctionType.Copy)


            # ------------------------------------------------------------
            def conv_layer(ht_d, w2_s, in_c, out_c, root_nm,
                           hout_nm, src_view, build_A, gather_tbl=None,
                           stream_xs=None, on_window=None):
                """One NNConv layer; writes node-major relu output into
                hout_nm ([P, NT_NODE*out_c], window w at cols [w*out_c:]).

                gather_tbl: None, or the padded DRAM h1 table
                [NCORES*NPAD, TBL_ROW]; h1[src] rows are then fetched with
                one dma_gather per GATHER_G-tile group (int16 idxs, signed
                rebase to TBL_BASE).
                stream_xs: None, or dram ap [P, T*NF] to stream per-chunk
                (conv1's x[src] path).
                on_window(w): called after window w's output is written.
                """
                # group lookup: tile t -> (group tile_lo, srcg16 col)
                g_of_t = {}
                for (g0, g1, col) in groups:
                    for t in range(g0, g1):
                        g_of_t[t] = (g0, g1, col)
                g_tiles = {}   # g0 -> live gathered tile
                for ci, (wlo, whi) in enumerate(chunks):
                    clo, chi = tile_start[wlo], tile_start[whi]
                    ct = chi - clo
                    if ct > 0:
                        ht_c = sp.tile([P, (CHUNK_T_MAX // 4) * P], BF16,
                                       tag="ht_c", bufs=3)
                        nc.sync.dma_start(
                            ht_c[:, :(ct // 4) * P],
                            ht_d[:, (clo // 4) * P:(chi // 4) * P])
                    xs_c = None
                    if stream_xs is not None and ct > 0:
                        xs_c = sp.tile([P, CHUNK_T_MAX * NF], BF16,
                                       tag="xs_c", bufs=3)
                        nc.sync.dma_start(
                            xs_c[:, :ct * NF],
                            stream_xs[:, clo * NF:chi * NF])
                    if gather_tbl is not None and ct > 0:
                        g_tiles.clear()
                        for gi, (g0, g1, col) in enumerate(groups):
                            if g0 < clo or g0 >= chi:
                                continue
                            gt = g1 - g0
                            n = gt * P
                            g_c = sp.tile([P, GATHER_G, TBL_ROW], BF16,
                                          tag="g_c", bufs=3)
                            nc.gpsimd.dma_gather(
                                out_ap=g_c[:, :gt, :],
                                in_ap=gather_tbl[TBL_BASE:, :],  # incl fence
                                idxs_ap=srcg16_s[:, col:col + gt * 8],
                                num_idxs=n, num_idxs_reg=n,
                                elem_size=TBL_ROW,
                                queue_num=gi % 4)
                            g_tiles[g0] = g_c
                    for w in range(wlo, whi):
                        nw = int(tiles_w[w])
                        if nw > 0:
                            unred = pagg.tile([P, 512], F32, tag="unred")

                        def src_of(t, tl):
                            if gather_tbl is not None:
                                g0, _, _ = g_of_t[t]
                                return g_tiles[g0][:, t - g0, 0:in_c]
                            if xs_c is not None:
                                return xs_c[:, tl * NF:(tl + 1) * NF]
                            return src_view(t, tl)

                        # ACT-path tiles whose successor is also ACT-path,
                        # same window + (conv2) same gather group, are
                        # MULT-PAIRED: two per-tile ACT evacs land in one
                        # [P,2,512] SBUF tile, ONE DVE op multiplies both
                        # (amortizes DVE fixed cost). PSUM stays per-tile.
                        def is_direct(t):
                            return t % 9 in (4, 8)

                        def same_grp(t):
                            if gather_tbl is None:
                                return True
                            return g_of_t[t][0] == g_of_t[t + 1][0]

                        ti = 0
                        while ti < nw:
                            t = int(tile_start[w]) + ti
                            tl = t - clo
                            paired = (ti + 1 < nw and not is_direct(t)
                                      and not is_direct(t + 1)
                                      and same_grp(t))
                            nt_u = 2 if paired else 1
                            we_u, A_u = [], []
                            for u in range(nt_u):
                                g4 = (t + u) % 4
                                tlu = tl + u
                                we = ppw.tile([P, 512], F32, tag="we",
                                              bufs=4)
                                nc.tensor.matmul(
                                    we[:],
                                    lhsT=ht_c[32 * g4:32 * (g4 + 1),
                                              (tlu // 4) * P:
                                              (tlu // 4 + 1) * P],
                                    rhs=w2_s[32 * g4:32 * (g4 + 1), :],
                                    start=True, stop=True,
                                    tile_position=(32 * g4, 0))
                                we_u.append(we)
                                A = Acache[:, (t + u) * WIN:
                                           (t + u + 1) * WIN]
                                if build_A:
                                    nc.vector.tensor_scalar(
                                        out=A, in0=iota128_s[:],
                                        scalar1=dstrel_s[:, t + u:t + u + 1],
                                        scalar2=None,
                                        op0=mybir.AluOpType.is_equal)
                                A_u.append(A)
                            tmp = wp.tile([P, nt_u, out_c, in_c], BF16,
                                          tag="tmp2" if paired else "tmp",
                                          bufs=2 if paired else 3)
                            if paired:
                                we_sb = wp.tile([P, 2, 512], BF16,
                                                tag="we_sb2", bufs=2)
                                for u in range(2):
                                    nc.scalar.activation(
                                        out=we_sb[:, u, :],
                                        in_=we_u[u][:],
                                        func=mybir.ActivationFunctionType
                                        .Copy)
                                if gather_tbl is not None:
                                    g0 = g_of_t[t][0]
                                    sp2 = g_tiles[g0][:, t - g0:t - g0 + 2,
                                                      0:in_c]
                                elif xs_c is not None:
                                    sp2 = xs_c[:, tl * NF:(tl + 2) * NF] \
                                        .rearrange("p (u i) -> p u i", u=2)
                                else:
                                    raise AssertionError("unreachable")
                                nc.vector.tensor_tensor(
                                    out=tmp[:],
                                    in0=we_sb[:].rearrange(
                                        "p u (o i) -> p u o i",
                                        o=out_c, i=in_c),
                                    in1=sp2[:, :, None, :].broadcast_to(
                                        [P, 2, out_c, in_c]),
                                    op=mybir.AluOpType.mult)
                            else:
                                src_b = src_of(t, tl)[:, None, :] \
                                    .broadcast_to([P, out_c, in_c])
                                if is_direct(t):
                                    # DVE reads We straight from PSUM:
                                    # fused evac+multiply, frees ACT
                                    we3 = we_u[0][:].rearrange(
                                        "p (o i) -> p o i",
                                        o=out_c, i=in_c)
                                else:
                                    we_sb1 = wp.tile([P, 512], BF16,
                                                     tag="we_sb", bufs=3)
                                    nc.scalar.activation(
                                        out=we_sb1[:], in_=we_u[0][:],
                                        func=mybir.ActivationFunctionType
                                        .Copy)
                                    we3 = we_sb1[:].rearrange(
                                        "p (o i) -> p o i",
                                        o=out_c, i=in_c)
                                nc.vector.tensor_tensor(
                                    out=tmp[:, 0], in0=we3, in1=src_b,
                                    op=mybir.AluOpType.mult)
                            # scatter UNREDUCED: unred += A.T @ tmp
                            for u in range(nt_u):
                                nc.tensor.matmul(
                                    unred[:], lhsT=A_u[u],
                                    rhs=tmp[:, u].rearrange(
                                        "p o i -> p (o i)"),
                                    start=(ti + u == 0),
                                    stop=(ti + u == nw - 1))
                            ti += nt_u
                        # combine: h = relu(reduce_i(unred) + root)
                        ocol = slice(w * out_c, (w + 1) * out_c)
                        if nw > 0:
                            r = wp.tile([P, out_c], F32, tag="r")
                            nc.vector.tensor_reduce(
                                out=r[:],
                                in_=unred[:].rearrange(
                                    "p (o i) -> p o i", o=out_c, i=in_c),
                                axis=mybir.AxisListType.X,
                                op=mybir.AluOpType.add)
                            s = wp.tile([P, out_c], F32, tag="s")
                            nc.vector.tensor_tensor(
                                out=s[:], in0=r[:], in1=root_nm[:, ocol],
                                op=mybir.AluOpType.add)
                            if in_c == NF:
                                # conv1: relu on DVE (ACT is the busier
                                # engine); conv2's stays on ACT
                                nc.vector.tensor_scalar(
                                    out=hout_nm[:, ocol], in0=s[:],
                                    scalar1=0.0, scalar2=None,
                                    op0=mybir.AluOpType.max)
                            else:
                                nc.scalar.activation(
                                    out=hout_nm[:, ocol], in_=s[:],
                                    func=mybir.ActivationFunctionType.Relu)
                        else:
                            nc.scalar.activation(
                                out=hout_nm[:, ocol],
                                in_=root_nm[:, ocol],
                                func=mybir.ActivationFunctionType.Relu)
                        if on_window is not None:
                            on_window(w)

            def run_once(rep):
                # DRAM internals for collectives (fresh per rep: Shared
                # DRAM may only be written by a single instruction)
                ag_in = dp.tile([NPAD, 32], BF16, tag=f"ag_in{rep}")
                # one Shared tensor per AG chunk (Shared DRAM allows only a
                # single writer inst); each chunk is then spread into the
                # 256B-padded gather table by a DRAM->DRAM copy
                # +1 fence row (never gathered: max idx reaches 50175).
                # Rows < TBL_BASE are read via negative idx offsets, which
                # the slice-view dep tracker can't see -- the fence copy
                # below makes the chunk-0 -> gather ordering explicit.
                tblpad = dp.tile([NCORES * NPAD + 1, TBL_ROW], BF16,
                                 tag=f"tblpad{rep}")
                ag_cc = []
                for c in range(len(AG_SPLITS) - 1):
                    rows = (AG_SPLITS[c + 1] - AG_SPLITS[c]) * WIN
                    ag_cc_c = dp.tile([NCORES * rows, 32], BF16,
                                      tag=f"ag_cc{rep}_{c}",
                                      addr_space="Shared", name=f"ag_cc{rep}_{c}")
                    ag_cc.append(ag_cc_c)
                ar_in = dp.tile([64, 16], F32, tag=f"ar_in{rep}")
                ar_out = dp.tile([NCORES * 64, 16], F32, tag=f"ar_out{rep}",
                                 addr_space="Shared")

                # ---- conv1, with h1 AllGather chunks fired as soon as the
                # windows they cover are done (overlaps collective w/ compute)
                ag_base = 0

                def fire_ag_chunk(c):
                    nonlocal ag_base
                    lo, hi = AG_SPLITS[c] * WIN, AG_SPLITS[c + 1] * WIN
                    rows = hi - lo
                    nc.gpsimd.dma_start(
                        ag_in[lo:hi, :].rearrange("(t p) f -> p t f", p=P),
                        h1nm[:, AG_SPLITS[c] * 32:AG_SPLITS[c + 1] * 32]
                        .rearrange("p (t f) -> p t f", f=32))
                    if SKIP_COLLECTIVES:
                        nc.sync.dma_start(
                            ag_cc[c][:rows, :], ag_in[lo:hi, :])
                    else:
                        nc.gpsimd.collective_compute(
                            "AllGather", mybir.AluOpType.bypass,
                            replica_groups=[list(range(NCORES))],
                            ins=[ag_in[lo:hi, :].opt()],
                            outs=[ag_cc[c][:].opt()])
                    nc.sync.dma_start(
                        tblpad[ag_base:ag_base + NCORES * rows, 0:32],
                        ag_cc[c][:])
                    if ag_base + NCORES * rows <= TBL_BASE:
                        # chunk fully below the gathers' visible slice:
                        # re-copy its first row into the fence row so the
                        # gather's input view carries a dependency on it
                        nc.sync.dma_start(
                            tblpad[NCORES * NPAD:NCORES * NPAD + 1, 0:32],
                            tblpad[ag_base:ag_base + 1, 0:32])
                    ag_base += NCORES * rows

                def on_window1(w):
                    for c in range(len(AG_SPLITS) - 1):
                        if w == AG_SPLITS[c + 1] - 1:
                            fire_ag_chunk(c)

                conv_layer(ht1_d, w2a_s, NF, 32, root1nm[:], h1nm,
                           None, build_A=(rep == 0), stream_xs=xs_d,
                           on_window=on_window1)

                # h1 transposes + conv2 root table (overlaps the AllGather)
                # transpose/root2 evacs on DVE: they overlap the AllGather,
                # where ACT is the busier engine in steady state
                for nt in range(NT_NODE):
                    tp = pp.tile([32, P], F32, tag="aux")
                    nc.tensor.transpose(
                        tp[:], in_=h1nm[:, nt * 32:(nt + 1) * 32],
                        identity=ident_s[:, :])
                    nc.vector.tensor_copy(
                        h1T[:32, nt * P:(nt + 1) * P], tp[:])
                for nt in range(NT_NODE):
                    rps = pp.tile([P, 16], F32, tag="aux")
                    nc.tensor.matmul(
                        rps[:], lhsT=h1T[:, nt * WIN:(nt + 1) * WIN],
                        rhs=root2_s[:], start=True, stop=True)
                    nc.vector.tensor_copy(
                        root2nm[:, nt * 16:(nt + 1) * 16], rps[:])
                if DEBUG_TAPS:
                    nc.sync.dma_start(taps["t_h1nm"], h1nm[:])

                # ---- conv2 with global_add_pool fused into the window
                # loop: per-window one-hot matmul lands in an aux PSUM tile
                # and is accumulated into SBUF g_acc by DVE, so the final
                # AllGather can start right after the last window
                nc.vector.memset(g_acc[:], 0.0)

                def pool_win(w):
                    B = wp.tile([P, 64], F32, tag="B", bufs=2)
                    nc.vector.tensor_scalar(
                        out=B[:], in0=iota64_s[:],
                        scalar1=blocal_s[:, w:w + 1], scalar2=None,
                        op0=mybir.AluOpType.is_equal)
                    paux = pp.tile([64, 16], F32, tag="aux")
                    nc.tensor.matmul(
                        paux[:], lhsT=B[:],
                        rhs=h2nm[:, w * 16:(w + 1) * 16],
                        start=True, stop=True)
                    nc.vector.tensor_tensor(
                        out=g_acc[:], in0=g_acc[:], in1=paux[:],
                        op=mybir.AluOpType.add)

                conv_layer(ht2_d, w2b_s, 32, 16, root2nm[:],
                           h2nm, None, build_A=False, gather_tbl=tblpad[:],
                           on_window=pool_win)

                if DEBUG_TAPS:
                    nc.sync.dma_start(taps["t_g"], g_acc[:])
                nc.sync.dma_start(ar_in[:], g_acc[:])
                if USE_NEW_TAIL:
                    if SKIP_COLLECTIVES:
                        for c in range(NCORES):
                            nc.sync.dma_start(
                                ar_out[64 * c:64 * (c + 1), :], ar_in[:])
                    else:
                        # AllGather [64,16] -> [8*64,16], then sum the 8
                        # slices on DVE: ~2x cheaper than AllReduce
                        nc.gpsimd.collective_compute(
                            "AllGather", mybir.AluOpType.bypass,
                            replica_groups=[list(range(NCORES))],
                            ins=[ar_in[:].opt()], outs=[ar_out[:].opt()])
                    g8 = wp.tile([64, NCORES, 16], F32, tag="g8")
                    nc.sync.dma_start(
                        g8[:], ar_out[:].rearrange("(c p) f -> p c f", p=64))
                    g_r = wp.tile([64, 16], F32, tag="g_r")
                    nc.vector.tensor_reduce(
                        out=g_r[:],
                        in_=g8[:].rearrange("p c f -> p f c"),
                        axis=mybir.AxisListType.X,
                        op=mybir.AluOpType.add)
                else:
                    if SKIP_COLLECTIVES:
                        nc.sync.dma_start(ar_out[:64, :], ar_in[:])
                    else:
                        nc.gpsimd.collective_compute(
                            "AllReduce", mybir.AluOpType.add,
                            replica_groups=[list(range(NCORES))],
                            ins=[ar_in[:].opt()],
                            outs=[ar_out[:64, :].opt()])
                    g_r = wp.tile([64, 16], F32, tag="g_r")
                    nc.sync.dma_start(g_r[:], ar_out[:64, :])

                # ---- head: y = relu(g@fc1+b)@out_w + out_b
                gT_ps = pp.tile([16, 64], F32, tag="aux")
                nc.tensor.transpose(gT_ps[:], in_=g_r[:], identity=ident_s[:64, :64])
                gT_s = wp.tile([16, 64], F32, tag="gT_s")
                nc.vector.tensor_copy(gT_s[:], gT_ps[:])
                o1 = pp.tile([32, 64], F32, tag="aux")
                nc.tensor.matmul(o1[:], lhsT=fc1w_s[:], rhs=gT_s[:],
                                 start=True, stop=True)
                r1 = wp.tile([32, 64], F32, tag="r1")
                nc.scalar.activation(out=r1[:], in_=o1[:],
                                     func=mybir.ActivationFunctionType.Relu,
                                     bias=fc1b_s[:])
                o2 = pp.tile([1, 64], F32, tag="aux")
                nc.tensor.matmul(o2[:], lhsT=outw_s[:], rhs=r1[:],
                                 start=True, stop=True)
                ys = wp.tile([1, 64], F32, tag="ys")
                nc.vector.tensor_scalar(out=ys[:], in0=o2[:],
                                        scalar1=outb_s[:], scalar2=None,
                                        op0=mybir.AluOpType.add)
                nc.sync.dma_start(y_d, ys[:])

            for _rep in range(reps):
                run_once(_rep)

    nc.compile()
    nc.m = get_hw_module(nc.m)
    return nc


# --------------------------------------------------------------------------
def kernel(**inputs):
    sched, per_core, shared = _host_prep(inputs)
    key = sched
    if key not in _cache:
        _cache[key] = _build_program(sched)
    nc = _cache[key]

    in_maps = []
    for k in range(NCORES):
        m = dict(shared)
        m.update(per_core[k])
        m = {n: np.ascontiguousarray(v) for n, v in m.items()}
        in_maps.append(m)

    res = bass_utils.run_bass_kernel_spmd(nc, in_maps,
                                          core_ids=list(range(NCORES)))
    y = np.asarray(res.results[0]["y"], np.float32).reshape(64, 1)
    return y


